# revision 20
# baseline (speedup 1.0000x reference)
"""Trainium2 Bass kernel for nn_Attention_Encode (B=4, N=2048, DIM=1024, H=16, DH=64).

Sharding: 16 heads -> 8 cores x 2 heads (tensor parallel). Each core computes
  ztu_g = W_g @ ZT^T          (its 128 output channels = 2 heads)
  attention per (batch, head) with Q=K=V=ztu
  partial_out = ssa_g @ W_g   (row-sharded output projection)
Host sums the 8 partials (the all-reduce step of a row-sharded projection).

Optimizations over the original baseline:
- QK pairs: the two heads' K=64 QK matmuls run CONCURRENTLY on the PE array
  via row tiling (head A rows 0:64, head B rows 64:128 of a merged ztuT) --
  one 512-cycle span computes both heads' scores.
- SYM: Q=K=V makes each head's score matrix symmetric. Per 512-query block J
  only key tiles i >= 4J are computed/exp'd; tiles above the diagonal
  super-block are produced by transpose-DMA "mirror strips" from already
  exp'd rows, emitted a block early so the transfers hide under compute.
  AV accumulates key tiles in descending order so mirror tiles are needed
  last. This removes ~40% of the ACT exp work (the real co-bottleneck).
- zin loads are one DMA per 512-token chunk; final stores go through the
  gpsimd SWDGE ring to keep the sync ring free for mirror strips.
"""
import sys

for _p in ('/opt/trn_rl_repo',):
    if _p not in sys.path:
        sys.path.insert(0, _p)

from contextlib import ExitStack

import numpy as np
import ml_dtypes

import concourse.bacc as bacc
import concourse.mybir as mybir
import concourse.tile as tile
from concourse.bass_utils import run_bass_kernel_spmd
from concourse.masks import make_identity

B, N, C = 4, 2048, 1024          # batch, seq, model dim
KP, DH, HPER = 128, 64, 2        # per-core channels, head dim, heads per core
NQB = 512                        # query block
NKT = 128                        # key tile
NTB = N // NKT                   # 16 key tiles per batch
NJB = N // NQB                   # 4 query blocks per batch
SCALE = DH ** -0.5               # 0.125
BF = mybir.dt.bfloat16
F32 = mybir.dt.float32
F32R = mybir.dt.float32r
SYM = True                       # exploit score-matrix symmetry

_CACHE = {}


def _build_kernel(dumps=False):
    nc = bacc.Bacc("TRN2", target_bir_lowering=False, debug=False)
    ztt = nc.dram_tensor("ztt", [B, C, N], BF, kind="ExternalInput").ap()
    wgt = nc.dram_tensor("wgt", [C, KP], BF, kind="ExternalInput").ap()   # W_g^T
    wg = nc.dram_tensor("wg", [KP, C], BF, kind="ExternalInput").ap()     # W_g
    out = nc.dram_tensor("out", [B * N, C], F32, kind="ExternalOutput").ap()
    dump_aps = None
    if dumps:
        dump_aps = {
            "ztd": nc.dram_tensor("ztd", [128, B * N], BF,
                                  kind="ExternalOutput").ap(),
            "znd": nc.dram_tensor("znd", [128, NTB, HPER, 66], BF,
                                  kind="ExternalOutput").ap(),
            "exd": nc.dram_tensor("exd", [128, NTB, HPER, N], BF,
                                  kind="ExternalOutput").ap(),
        }

    with tile.TileContext(nc) as tc, ExitStack() as ctx:
        _body(ctx, tc, ztt, wgt, wg, out, dump_aps)
    nc.compile()
    return nc


def _body(ctx, tc, ztt, wgt, wg, out, dump_aps=None):
    nc = tc.nc
    singles = ctx.enter_context(tc.tile_pool(name="singles", bufs=1))
    zin_pool = ctx.enter_context(tc.tile_pool(name="zin", bufs=2))
    ztn_pool = ctx.enter_context(tc.tile_pool(name="ztn", bufs=2))
    sc_pool = ctx.enter_context(tc.tile_pool(name="sc", bufs=2, space="PSUM"))
    av_pool = ctx.enter_context(tc.tile_pool(name="av", bufs=2, space="PSUM"))
    p2_pool = ctx.enter_context(tc.tile_pool(name="p2", bufs=2, space="PSUM"))
    sn_pool = ctx.enter_context(tc.tile_pool(name="sn", bufs=4))
    rc_pool = ctx.enter_context(tc.tile_pool(name="rc", bufs=4))
    ps_pool = ctx.enter_context(tc.tile_pool(name="ps", bufs=4))

    # ---- persistent SBUF ----
    wgt_sb = singles.tile([128, 8, KP], BF)            # [c-in-tile, ci, k]
    nc.sync.dma_start(out=wgt_sb, in_=wgt.rearrange("(ci p) k -> p ci k", p=128))
    wg_sb = singles.tile([KP, C], BF)
    nc.sync.dma_start(out=wg_sb, in_=wg)
    ident = singles.tile([128, 128], BF)
    make_identity(nc, ident)
    self_f = singles.tile([128, 128], F32)
    nc.vector.memset(self_f, 0.0)
    nc.vector.memset(self_f[0:1, 0:64], 1.0)
    nc.vector.memset(self_f[32:33, 64:128], 1.0)
    sel = singles.tile([128, 128], F32R)               # den -> per-head row broadcast
    nc.vector.tensor_copy(out=sel, in_=self_f)
    dn = singles.tile([128, NQB], F32R)                # dens: head A row 0, head B row 32
    nc.vector.memset(dn[:].bitcast(F32), 0.0)
    # Merged per-head ztu^T: head A rows 0:64, head B rows 64:128. QK uses
    # K=64 row-tiled matmul pairs that run concurrently on the two array halves.
    ztuT = singles.tile([128, B * N], BF)
    # Full EX (exp'd scores, transposed layout [keys, q]) for ONE batch,
    # both heads: [p=key-in-tile, kt, head, q].
    EX = singles.tile([128, NTB, HPER, N], BF)

    # ---- proj1: ztuT = W_g @ ZT^T, one 512-token chunk per DMA ----
    def load_zin_chunk(b, jn):
        # SWDGE ring: keeps these off the sync ring, where they'd queue
        # behind the mirror-strip bursts and stall proj1 (~26us/run).
        z = zin_pool.tile([128, 8, NQB], BF, tag="zin")
        nc.gpsimd.dma_start(
            out=z,
            in_=ztt[b, :, jn * NQB:(jn + 1) * NQB].rearrange(
                "(ci p) n -> p ci n", p=128))
        return z

    def proj1_chunk(b, jn, zc):
        p1 = sc_pool.tile([128, HPER, NQB], F32, tag="sc")
        p1v = p1[:, 0, :]
        for ci in range(8):
            nc.tensor.matmul(
                p1v, lhsT=wgt_sb[:, ci, :], rhs=zc[:, ci, :],
                start=(ci == 0), stop=(ci == 7),
            )
        nc.vector.tensor_copy(
            out=ztuT[:, b * N + jn * NQB: b * N + (jn + 1) * NQB], in_=p1v)

    # ztuN: v in natural layout per key tile + ones column for the softmax
    # denominator: [p=token-in-tile, kt, head, 0:64]=v, [...,64]=1.
    # Built with PE transposes (proven path).
    def make_ztuN(b):
        zt = ztn_pool.tile([128, NTB, HPER, 66], BF, tag="ztn", name=f"ztn{b}")
        nc.gpsimd.memset(zt[:, :, :, 64:65], 1.0)
        for nt in range(NTB):
            pt = av_pool.tile([128, NQB], BF, tag="av", name="pt")
            nc.tensor.transpose(
                pt[:, 0:128],
                ztuT[:, b * N + nt * NKT: b * N + (nt + 1) * NKT],
                ident)
            for h in range(HPER):
                nc.vector.tensor_copy(
                    out=zt[:, nt, h, 0:64],
                    in_=pt[:, h * 64:(h + 1) * 64])
        return zt

    # ---- attention ----
    def mirror_rows(J):
        # key tiles below this are mirrored rather than computed; capped at 8
        # so block 3's strips only need columns 0:1024 (ready a block early,
        # keeping the transfers off the critical path).
        return min(4 * J, 8) if SYM else 0

    def emit_strips(b, J):
        # mirror strips for block J: row jt (cols < m*128) -> column jt
        # (key tiles 0..m-1).
        m = mirror_rows(J)
        for jt in range(4 * J, 4 * J + 4):
            for h in range(HPER):
                nc.sync.dma_start_transpose(
                    out=EX[:, 0:m, h, jt * NKT:(jt + 1) * NKT],
                    in_=EX[:, jt, h, 0:m * NKT])

    def qk_unit(b, J, i):
        # one direct row: QK pair (row-tiled, both heads concurrent) + exp
        q0 = b * N + J * NQB
        k0 = b * N + i * NKT
        sc = sc_pool.tile([128, HPER, NQB], F32, tag="sc")
        nc.tensor.matmul(sc[:, 0, :], lhsT=ztuT[0:64, k0:k0 + NKT],
                         rhs=ztuT[0:64, q0:q0 + NQB], start=True, stop=True)
        nc.tensor.matmul(sc[:, 1, :], lhsT=ztuT[64:128, k0:k0 + NKT],
                         rhs=ztuT[64:128, q0:q0 + NQB], start=True, stop=True)
        nc.scalar.activation(
            out=EX[:, i, :, J * NQB:(J + 1) * NQB], in_=sc,
            func=mybir.ActivationFunctionType.Exp, scale=SCALE)

    def finish_norm(b, jq, avs):
        # softmax denominators -> per-head broadcast -> reciprocal -> scale
        nc.vector.tensor_copy(out=dn[0:1, :], in_=avs[0][DH:DH + 1, :])
        nc.vector.tensor_copy(out=dn[32:33, :], in_=avs[1][DH:DH + 1, :])
        bc = p2_pool.tile([128, NQB], F32, tag="p2", name="bc")
        bcv = bc[:, 0:NQB]
        nc.tensor.matmul(bcv, lhsT=sel, rhs=dn, start=True, stop=True)
        rc = rc_pool.tile([128, NQB], F32, tag="rc")
        nc.vector.reciprocal_approx_fast(out=rc, in_=bcv)
        sn = sn_pool.tile([128, NQB], BF)
        nc.vector.tensor_tensor(
            out=sn[0:64, :], in0=avs[0][0:DH, :], in1=rc[0:64, :],
            op=mybir.AluOpType.mult)
        nc.vector.tensor_tensor(
            out=sn[64:128, :], in0=avs[1][0:DH, :], in1=rc[64:128, :],
            op=mybir.AluOpType.mult)
        return sn

    def finish_proj2_t(b, jq, sn, t):
        # one 128-row unit of proj2; stores ride the SWDGE ring.
        p2s = ps_pool.tile([128, 2, 512], F32, tag="ps")
        for ch in range(2):
            p2 = p2_pool.tile([128, NQB], F32, tag="p2", name="p2")
            p2v = p2[:, 0:512]
            nc.tensor.matmul(
                p2v, lhsT=sn[:, t * 128:(t + 1) * 128],
                rhs=wg_sb[:, ch * 512:(ch + 1) * 512],
                start=True, stop=True)
            nc.vector.tensor_copy(out=p2s[:, ch, :], in_=p2v)
        r0 = b * N + jq * NQB + t * 128
        nc.gpsimd.dma_start(out=out[r0:r0 + 128, :], in_=p2s)

    def finish_proj2(b, jq, sn):
        for t in range(NQB // 128):
            finish_proj2_t(b, jq, sn, t)

    # ---- main schedule ----
    state = {"pending": None, "sn": None}

    def flush_norm():
        if state["pending"] is not None:
            state["sn"] = (state["pending"][0], state["pending"][1],
                           finish_norm(*state["pending"]))
            state["pending"] = None

    def flush_proj2():
        if state["sn"] is not None:
            finish_proj2(*state["sn"])
            state["sn"] = None

    def attention_block(b, J, zt, filler):
        # Emission order is tuned for the in-order PE queue: the ACT exp
        # stream (1.1us/row) governs the direct phase, so dependency-free PE
        # work (previous block's norm/proj2, proj1 filler, mirror AVs) is
        # emitted BETWEEN the ACT-gated QK/AV units to fill the gaps.
        i0 = mirror_rows(J)
        navs = [0]
        avs = [None]

        def emit_av(i):
            if avs[0] is None:
                avs[0] = [av_pool.tile([128, NQB], F32, tag="av",
                                       name=f"av{h}") for h in range(HPER)]
            for h in range(HPER):
                nc.tensor.matmul(
                    avs[0][h][0:65, :], lhsT=zt[:, i, h, 0:65],
                    rhs=EX[:, i, h, J * NQB:(J + 1) * NQB],
                    start=(navs[0] == 0), stop=(navs[0] == NTB - 1))
            navs[0] += 1

        for k, i in enumerate(range(i0, NTB)):
            qk_unit(b, J, i)
            # strips as soon as their source rows are exp'd (a block+ early)
            if SYM and J == 0 and i == 7:
                emit_strips(b, 1)
            elif SYM and J == 1 and i == 11:
                emit_strips(b, 2)
            elif SYM and J == 1 and i == NTB - 1:
                emit_strips(b, 3)
            if k == 0:
                flush_norm()
            elif k == 1 and filler is not None:
                filler(J)
            elif k == 2:
                flush_proj2()
            elif k == 3:
                for m in range(0, i0):      # mirror AVs: dep-free gap filler
                    emit_av(m)
            if k >= 1:
                emit_av(i - 1)
        emit_av(NTB - 1)
        state["pending"] = (b, J, avs[0])

    def attention_batch(b, zt, filler=None):
        for J in range(NJB):
            attention_block(b, J, zt, filler)

    # PE warm-up spin
    warm = p2_pool.tile([128, NQB], F32, tag="p2", name="warm")
    for _ in range(160):
        nc.tensor.matmul(warm[:, 0:32], lhsT=ident, rhs=ident[:, 0:32],
                         start=True, stop=True)
    del warm

    chunks = {(0, 0): load_zin_chunk(0, 0), (0, 1): load_zin_chunk(0, 1)}

    def get_chunk(b, jn):
        zc = chunks.pop((b, jn))
        nj, nb = jn + 2, b
        if nj >= NJB:
            nj, nb = nj - NJB, b + 1
        if nb < B and (nb, nj) not in chunks:
            chunks[(nb, nj)] = load_zin_chunk(nb, nj)
        return zc

    for jn in range(NJB):
        proj1_chunk(0, jn, get_chunk(0, jn))
    zts = {0: make_ztuN(0)}

    for b in range(B):
        if b + 1 < B:
            def filler(J, b=b):
                proj1_chunk(b + 1, J, get_chunk(b + 1, J))
                if J == NJB - 1:
                    zts[b + 1] = make_ztuN(b + 1)
        else:
            filler = None
        zt_b = zts.pop(b)
        attention_batch(b, zt_b, filler)
        if dump_aps is not None and b == 0:
            nc.sync.dma_start(out=dump_aps["ztd"], in_=ztuT)
            nc.sync.dma_start(out=dump_aps["znd"], in_=zt_b)
            nc.sync.dma_start(out=dump_aps["exd"], in_=EX)
    flush_norm()
    flush_proj2()


def _get_nc():
    if "nc" not in _CACHE:
        _CACHE["nc"] = _build_kernel()
    return _CACHE["nc"]


def kernel(ZT: np.ndarray, W: np.ndarray) -> np.ndarray:
    ZT = np.asarray(ZT, dtype=np.float32)
    W = np.asarray(W, dtype=np.float32)
    ztt = np.ascontiguousarray(ZT.transpose(0, 2, 1)).astype(ml_dtypes.bfloat16)
    in_maps = []
    for c in range(8):
        wgf = W[c * KP:(c + 1) * KP, :]
        in_maps.append({
            "ztt": ztt,
            "wgt": np.ascontiguousarray(wgf.T).astype(ml_dtypes.bfloat16),
            "wg": np.ascontiguousarray(wgf).astype(ml_dtypes.bfloat16),
        })
    nc = _get_nc()
    res = run_bass_kernel_spmd(nc, in_maps, core_ids=list(range(8)))
    acc = np.zeros((B * N, C), dtype=np.float32)
    for r in res.results:
        acc += r["out"]
    return acc.reshape(B, N, C)


if __name__ == "__main__":
    rng = np.random.default_rng(0)
    zt = rng.standard_normal((B, N, C), dtype=np.float32)
    w = rng.standard_normal((KP * 8, C), dtype=np.float32) * C ** -0.5
    o = kernel(zt, w)
    print("out", o.shape, o.dtype, float(np.abs(o).mean()))


# revision 21
# speedup vs baseline: 1.0435x; 1.0435x over previous
"""Trainium2 Bass kernel for nn_Attention_Encode (B=4, N=2048, DIM=1024, H=16, DH=64).

Sharding: 16 heads -> 8 cores x 2 heads (tensor parallel). Each core computes
  ztu_g = W_g @ ZT^T          (its 128 output channels = 2 heads)
  attention per (batch, head) with Q=K=V=ztu
  partial_out = ssa_g @ W_g   (row-sharded output projection)
Host sums the 8 partials (the all-reduce step of a row-sharded projection).

Optimizations over the original baseline:
- QK pairs: the two heads' K=64 QK matmuls run CONCURRENTLY on the PE array
  via row tiling (head A rows 0:64, head B rows 64:128 of a merged ztuT) --
  one 512-cycle span computes both heads' scores.
- SYM: Q=K=V makes each head's score matrix symmetric. Per 512-query block J
  only key tiles i >= 4J are computed/exp'd; tiles above the diagonal
  super-block are produced by transpose-DMA "mirror strips" from already
  exp'd rows, emitted a block early so the transfers hide under compute.
  AV accumulates key tiles in descending order so mirror tiles are needed
  last. This removes ~40% of the ACT exp work (the real co-bottleneck).
- zin loads are one DMA per 512-token chunk; final stores go through the
  gpsimd SWDGE ring to keep the sync ring free for mirror strips.
"""
import sys

for _p in ('/opt/trn_rl_repo',):
    if _p not in sys.path:
        sys.path.insert(0, _p)

from contextlib import ExitStack

import numpy as np
import ml_dtypes

import concourse.bacc as bacc
import concourse.mybir as mybir
import concourse.tile as tile
from concourse.bass_utils import run_bass_kernel_spmd
from concourse.masks import make_identity

B, N, C = 4, 2048, 1024          # batch, seq, model dim
KP, DH, HPER = 128, 64, 2        # per-core channels, head dim, heads per core
NQB = 512                        # query block
NKT = 128                        # key tile
NTB = N // NKT                   # 16 key tiles per batch
NJB = N // NQB                   # 4 query blocks per batch
SCALE = DH ** -0.5               # 0.125
BF = mybir.dt.bfloat16
F32 = mybir.dt.float32
F32R = mybir.dt.float32r
SYM = True                       # exploit score-matrix symmetry

_CACHE = {}


def _build_kernel(dumps=False):
    nc = bacc.Bacc("TRN2", target_bir_lowering=False, debug=False)
    ztt = nc.dram_tensor("ztt", [B, C, N], BF, kind="ExternalInput").ap()
    wgt = nc.dram_tensor("wgt", [C, KP], BF, kind="ExternalInput").ap()   # W_g^T
    wg = nc.dram_tensor("wg", [KP, C], BF, kind="ExternalInput").ap()     # W_g
    out = nc.dram_tensor("out", [B * N, C], F32, kind="ExternalOutput").ap()
    dump_aps = None
    if dumps:
        dump_aps = {
            "ztd": nc.dram_tensor("ztd", [128, B * N], BF,
                                  kind="ExternalOutput").ap(),
            "znd": nc.dram_tensor("znd", [128, NTB, HPER, 66], BF,
                                  kind="ExternalOutput").ap(),
            "exd": nc.dram_tensor("exd", [128, NTB, HPER, N], BF,
                                  kind="ExternalOutput").ap(),
        }

    with tile.TileContext(nc) as tc, ExitStack() as ctx:
        _body(ctx, tc, ztt, wgt, wg, out, dump_aps)
    nc.compile()
    return nc


def _body(ctx, tc, ztt, wgt, wg, out, dump_aps=None):
    nc = tc.nc
    singles = ctx.enter_context(tc.tile_pool(name="singles", bufs=1))
    zin_pool = ctx.enter_context(tc.tile_pool(name="zin", bufs=2))
    ztn_pool = ctx.enter_context(tc.tile_pool(name="ztn", bufs=2))
    sc_pool = ctx.enter_context(tc.tile_pool(name="sc", bufs=2, space="PSUM"))
    av_pool = ctx.enter_context(tc.tile_pool(name="av", bufs=2, space="PSUM"))
    p2_pool = ctx.enter_context(tc.tile_pool(name="p2", bufs=2, space="PSUM"))
    sn_pool = ctx.enter_context(tc.tile_pool(name="sn", bufs=4))
    rc_pool = ctx.enter_context(tc.tile_pool(name="rc", bufs=4))
    ps_pool = ctx.enter_context(tc.tile_pool(name="ps", bufs=4))

    # ---- persistent SBUF ----
    wgt_sb = singles.tile([128, 8, KP], BF)            # [c-in-tile, ci, k]
    nc.sync.dma_start(out=wgt_sb, in_=wgt.rearrange("(ci p) k -> p ci k", p=128))
    wg_sb = singles.tile([KP, C], BF)
    nc.sync.dma_start(out=wg_sb, in_=wg)
    ident = singles.tile([128, 128], BF)
    make_identity(nc, ident)
    self_f = singles.tile([128, 128], F32)
    nc.vector.memset(self_f, 0.0)
    nc.vector.memset(self_f[0:1, 0:64], 1.0)
    nc.vector.memset(self_f[32:33, 64:128], 1.0)
    sel = singles.tile([128, 128], F32R)               # den -> per-head row broadcast
    nc.vector.tensor_copy(out=sel, in_=self_f)
    dn = singles.tile([128, NQB], F32R)                # dens: head A row 0, head B row 32
    nc.vector.memset(dn[:].bitcast(F32), 0.0)
    # Merged per-head ztu^T: head A rows 0:64, head B rows 64:128. QK uses
    # K=64 row-tiled matmul pairs that run concurrently on the two array halves.
    ztuT = singles.tile([128, B * N], BF)
    # Full EX (exp'd scores, transposed layout [keys, q]) for ONE batch,
    # both heads: [p=key-in-tile, kt, head, q].
    EX = singles.tile([128, NTB, HPER, N], BF)

    # ---- proj1: ztuT = W_g @ ZT^T, one 512-token chunk per DMA ----
    def load_zin_chunk(b, jn):
        # Scalar HWDGE ring (otherwise DMA-free): keeps these off the sync
        # ring, where they'd queue behind mirror-strip bursts and stall proj1.
        z = zin_pool.tile([128, 8, NQB], BF, tag="zin")
        nc.scalar.dma_start(
            out=z,
            in_=ztt[b, :, jn * NQB:(jn + 1) * NQB].rearrange(
                "(ci p) n -> p ci n", p=128))
        return z

    def proj1_chunk(b, jn, zc):
        p1 = sc_pool.tile([128, HPER, NQB], F32, tag="sc")
        p1v = p1[:, 0, :]
        for ci in range(8):
            nc.tensor.matmul(
                p1v, lhsT=wgt_sb[:, ci, :], rhs=zc[:, ci, :],
                start=(ci == 0), stop=(ci == 7),
            )
        nc.vector.tensor_copy(
            out=ztuT[:, b * N + jn * NQB: b * N + (jn + 1) * NQB], in_=p1v)

    # ztuN: v in natural layout per key tile + ones column for the softmax
    # denominator: [p=token-in-tile, kt, head, 0:64]=v, [...,64]=1.
    # Built with PE transposes (proven path).
    def make_ztuN(b):
        zt = ztn_pool.tile([128, NTB, HPER, 66], BF, tag="ztn", name=f"ztn{b}")
        nc.gpsimd.memset(zt[:, :, :, 64:65], 1.0)
        for nt in range(NTB):
            pt = av_pool.tile([128, NQB], BF, tag="av", name="pt")
            nc.tensor.transpose(
                pt[:, 0:128],
                ztuT[:, b * N + nt * NKT: b * N + (nt + 1) * NKT],
                ident)
            for h in range(HPER):
                nc.vector.tensor_copy(
                    out=zt[:, nt, h, 0:64],
                    in_=pt[:, h * 64:(h + 1) * 64])
        return zt

    # ---- attention ----
    def mirror_rows(J):
        # key tiles below this are mirrored rather than computed; capped at 8
        # so block 3's strips only need columns 0:1024 (ready a block early,
        # keeping the transfers off the critical path).
        return min(4 * J, 8) if SYM else 0

    def emit_strips(b, J):
        # mirror strips for block J: row jt (cols < m*128) -> column jt
        # (key tiles 0..m-1).
        m = mirror_rows(J)
        for jt in range(4 * J, 4 * J + 4):
            for h in range(HPER):
                nc.sync.dma_start_transpose(
                    out=EX[:, 0:m, h, jt * NKT:(jt + 1) * NKT],
                    in_=EX[:, jt, h, 0:m * NKT])

    def qk_unit(b, J, i):
        # one direct row: QK pair (row-tiled, both heads concurrent) + exp
        q0 = b * N + J * NQB
        k0 = b * N + i * NKT
        sc = sc_pool.tile([128, HPER, NQB], F32, tag="sc")
        nc.tensor.matmul(sc[:, 0, :], lhsT=ztuT[0:64, k0:k0 + NKT],
                         rhs=ztuT[0:64, q0:q0 + NQB], start=True, stop=True)
        nc.tensor.matmul(sc[:, 1, :], lhsT=ztuT[64:128, k0:k0 + NKT],
                         rhs=ztuT[64:128, q0:q0 + NQB], start=True, stop=True)
        nc.scalar.activation(
            out=EX[:, i, :, J * NQB:(J + 1) * NQB], in_=sc,
            func=mybir.ActivationFunctionType.Exp, scale=SCALE)

    def finish_norm(b, jq, avs):
        # softmax denominators -> per-head broadcast -> reciprocal -> scale
        nc.vector.tensor_copy(out=dn[0:1, :], in_=avs[0][DH:DH + 1, :])
        nc.vector.tensor_copy(out=dn[32:33, :], in_=avs[1][DH:DH + 1, :])
        bc = p2_pool.tile([128, NQB], F32, tag="p2", name="bc")
        bcv = bc[:, 0:NQB]
        nc.tensor.matmul(bcv, lhsT=sel, rhs=dn, start=True, stop=True)
        rc = rc_pool.tile([128, NQB], F32, tag="rc")
        nc.vector.reciprocal_approx_fast(out=rc, in_=bcv)
        sn = sn_pool.tile([128, NQB], BF)
        nc.vector.tensor_tensor(
            out=sn[0:64, :], in0=avs[0][0:DH, :], in1=rc[0:64, :],
            op=mybir.AluOpType.mult)
        nc.vector.tensor_tensor(
            out=sn[64:128, :], in0=avs[1][0:DH, :], in1=rc[64:128, :],
            op=mybir.AluOpType.mult)
        return sn

    def finish_proj2_t(b, jq, sn, t):
        # one 128-row unit of proj2; stores ride the SWDGE ring.
        p2s = ps_pool.tile([128, 2, 512], F32, tag="ps")
        for ch in range(2):
            p2 = p2_pool.tile([128, NQB], F32, tag="p2", name="p2")
            p2v = p2[:, 0:512]
            nc.tensor.matmul(
                p2v, lhsT=sn[:, t * 128:(t + 1) * 128],
                rhs=wg_sb[:, ch * 512:(ch + 1) * 512],
                start=True, stop=True)
            nc.vector.tensor_copy(out=p2s[:, ch, :], in_=p2v)
        r0 = b * N + jq * NQB + t * 128
        nc.gpsimd.dma_start(out=out[r0:r0 + 128, :], in_=p2s)

    def finish_proj2(b, jq, sn):
        for t in range(NQB // 128):
            finish_proj2_t(b, jq, sn, t)

    # ---- main schedule ----
    state = {"pending": None, "sn": None}

    def flush_norm():
        if state["pending"] is not None:
            state["sn"] = (state["pending"][0], state["pending"][1],
                           finish_norm(*state["pending"]))
            state["pending"] = None

    def flush_proj2():
        if state["sn"] is not None:
            finish_proj2(*state["sn"])
            state["sn"] = None

    def attention_block(b, J, zt, filler):
        # Emission order is tuned for the in-order PE queue: the ACT exp
        # stream (1.1us/row) governs the direct phase, so dependency-free PE
        # work (previous block's norm/proj2, proj1 filler, mirror AVs) is
        # emitted BETWEEN the ACT-gated QK/AV units to fill the gaps.
        i0 = mirror_rows(J)
        navs = [0]
        avs = [None]

        def emit_av(i):
            if avs[0] is None:
                avs[0] = [av_pool.tile([128, NQB], F32, tag="av",
                                       name=f"av{h}") for h in range(HPER)]
            for h in range(HPER):
                nc.tensor.matmul(
                    avs[0][h][0:65, :], lhsT=zt[:, i, h, 0:65],
                    rhs=EX[:, i, h, J * NQB:(J + 1) * NQB],
                    start=(navs[0] == 0), stop=(navs[0] == NTB - 1))
            navs[0] += 1

        for k, i in enumerate(range(i0, NTB)):
            qk_unit(b, J, i)
            # strips as soon as their source rows are exp'd (a block+ early)
            if SYM and J == 0 and i == 7:
                emit_strips(b, 1)
            elif SYM and J == 1 and i == 11:
                emit_strips(b, 2)
            elif SYM and J == 1 and i == NTB - 1:
                emit_strips(b, 3)
            if k == 0:
                flush_norm()
            elif k == 1 and filler is not None:
                filler(J)
            elif k == 2:
                flush_proj2()
            elif k == 3:
                for m in range(0, i0):      # mirror AVs: dep-free gap filler
                    emit_av(m)
            if k >= 1:
                emit_av(i - 1)
        emit_av(NTB - 1)
        state["pending"] = (b, J, avs[0])

    def attention_batch(b, zt, filler=None):
        for J in range(NJB):
            attention_block(b, J, zt, filler)

    # PE warm-up spin
    warm = p2_pool.tile([128, NQB], F32, tag="p2", name="warm")
    for _ in range(160):
        nc.tensor.matmul(warm[:, 0:32], lhsT=ident, rhs=ident[:, 0:32],
                         start=True, stop=True)
    del warm

    chunks = {(0, 0): load_zin_chunk(0, 0), (0, 1): load_zin_chunk(0, 1)}

    def get_chunk(b, jn):
        zc = chunks.pop((b, jn))
        nj, nb = jn + 2, b
        if nj >= NJB:
            nj, nb = nj - NJB, b + 1
        if nb < B and (nb, nj) not in chunks:
            chunks[(nb, nj)] = load_zin_chunk(nb, nj)
        return zc

    for jn in range(NJB):
        proj1_chunk(0, jn, get_chunk(0, jn))
    zts = {0: make_ztuN(0)}

    for b in range(B):
        if b + 1 < B:
            def filler(J, b=b):
                proj1_chunk(b + 1, J, get_chunk(b + 1, J))
                if J == NJB - 1:
                    zts[b + 1] = make_ztuN(b + 1)
        else:
            filler = None
        zt_b = zts.pop(b)
        attention_batch(b, zt_b, filler)
        if dump_aps is not None and b == 0:
            nc.sync.dma_start(out=dump_aps["ztd"], in_=ztuT)
            nc.sync.dma_start(out=dump_aps["znd"], in_=zt_b)
            nc.sync.dma_start(out=dump_aps["exd"], in_=EX)
    flush_norm()
    flush_proj2()


def _get_nc():
    if "nc" not in _CACHE:
        _CACHE["nc"] = _build_kernel()
    return _CACHE["nc"]


def kernel(ZT: np.ndarray, W: np.ndarray) -> np.ndarray:
    ZT = np.asarray(ZT, dtype=np.float32)
    W = np.asarray(W, dtype=np.float32)
    ztt = np.ascontiguousarray(ZT.transpose(0, 2, 1)).astype(ml_dtypes.bfloat16)
    in_maps = []
    for c in range(8):
        wgf = W[c * KP:(c + 1) * KP, :]
        in_maps.append({
            "ztt": ztt,
            "wgt": np.ascontiguousarray(wgf.T).astype(ml_dtypes.bfloat16),
            "wg": np.ascontiguousarray(wgf).astype(ml_dtypes.bfloat16),
        })
    nc = _get_nc()
    res = run_bass_kernel_spmd(nc, in_maps, core_ids=list(range(8)))
    acc = np.zeros((B * N, C), dtype=np.float32)
    for r in res.results:
        acc += r["out"]
    return acc.reshape(B, N, C)


if __name__ == "__main__":
    rng = np.random.default_rng(0)
    zt = rng.standard_normal((B, N, C), dtype=np.float32)
    w = rng.standard_normal((KP * 8, C), dtype=np.float32) * C ** -0.5
    o = kernel(zt, w)
    print("out", o.shape, o.dtype, float(np.abs(o).mean()))
